# revision 6
# baseline (speedup 1.0000x reference)
"""Trainium2 Bass kernel for the Involution module (B=4, C=64, H=W=128, K=7, G=4).

v3 = v2 (pixel-partition layout, stride-0 broadcast muls, PE accumulation)
plus:
  - only one x copy (dh=3, natural rows) is loaded from HBM; the 6
    row-shifted copies are built on-chip with partition-shifted
    SBUF->SBUF DMAs (cuts HBM input ~4x; the v2 trace showed the muls
    stalling ~6-8us per dh group on contended HBM loads).
  - k-loop runs dh=3 first so its muls start while the shifted copies
    are still being built.
  - conv/SiLU and the MAC loop are pipelined at pl-half granularity
    (cuts the serial prologue roughly in half); 4 shared PSUM
    accumulator banks + 4 conv banks fit PSUM exactly.
  - output returned as bf16 (halves output DMA).
"""

import os

os.environ.setdefault("JAX_PLATFORMS", "cpu")

import numpy as np
import ml_dtypes

import concourse.bacc as bacc
import concourse.tile as tile
import concourse.mybir as mybir
from concourse.bass_utils import run_bass_kernel_spmd

# Problem constants (hardcoded per harness contract).
B, C, H, W = 4, 64, 128, 128
K, G, GC = 7, 4, 16
KK = K * K
KO = KK * G  # 196
PAD = 3
BN_EPS = 1e-5

NPL = 64  # output cols per core (W-half)
XT_COLS = NPL + 2 * PAD  # 70 stored cols (halo included)
XT_F = XT_COLS * C  # 4480 free elems per dh-copy
QF = NPL * GC * G  # 4096 free elems of out
WQF = NPL * KO  # 12544 free elems of w'

DH_ORDER = [3, 2, 4, 1, 5, 0, 6]

USE_BF16 = True


def _dt():
    return mybir.dt.bfloat16 if USE_BF16 else mybir.dt.float32


def _npdt():
    return ml_dtypes.bfloat16 if USE_BF16 else np.float32


def build_bass():
    nc = bacc.Bacc(
        "TRN2",
        target_bir_lowering=False,
        debug=False,
        enable_asserts=False,
        num_devices=8,
    )
    DT = _dt()

    xc_d = nc.dram_tensor("xc", [C + 1, NPL * H], DT, kind="ExternalInput").ap()
    wt_d = nc.dram_tensor("wt", [C + 1, KO], DT, kind="ExternalInput").ap()
    id_d = nc.dram_tensor("ide", [128, 128 + 2 * PAD], DT, kind="ExternalInput").ap()
    x3_d = nc.dram_tensor("x3", [128, XT_F], DT, kind="ExternalInput").ap()
    out_d = nc.dram_tensor("out", [128, QF], DT, kind="ExternalOutput").ap()

    with tile.TileContext(nc) as tc:
        build_kernel(tc, xc_d, wt_d, id_d, x3_d, out_d)
    nc.compile()
    return nc


def build_kernel(tc, xc_d, wt_d, id_d, x3_d, out_d):
    from contextlib import ExitStack

    nc = tc.nc
    DT = _dt()
    f32 = mybir.dt.float32
    silu = mybir.ActivationFunctionType.Silu

    ctx = ExitStack()
    consts = ctx.enter_context(tc.tile_pool(name="consts", bufs=1))
    qpool = ctx.enter_context(tc.tile_pool(name="q", bufs=3))

    wt = consts.tile([C + 1, KO], DT)
    nc.sync.dma_start(out=wt, in_=wt_d)
    xc = consts.tile([C + 1, NPL * H], DT)
    # ide[p, p+3] = 1: slice [:, 3+d : 131+d] is the row-shift-by-d
    # matrix (out[m] = in[m+d], zeros where out of range); [:, 3:131]
    # is the plain identity used by the accumulate matmuls.
    ide = consts.tile([128, 128 + 2 * PAD], DT)
    nc.sync.dma_start(out=ide, in_=id_d)

    # xt[dh][r, (t, c)] = x[perm[c], r+dh-3, c0-3+t]; dh=3 from HBM, the
    # rest built by PE row-shift matmuls (boundary rows zero for free).
    xt = [None] * K
    for dh in range(K):
        t = consts.tile([128, XT_F], DT, tag="", name=f"xts{dh}")
        xt[dh] = t
    # Prioritized, chunked input loads: conv half 0 first (it gates the
    # whole pipeline), then x3, then conv half 1.
    HXC = NPL * H // 2
    nc.sync.dma_start(out=xc[:, 0:HXC], in_=xc_d[:, 0:HXC])
    for s in range(0, XT_F, 1536):
        e = min(XT_F, s + 1536)
        nc.sync.dma_start(out=xt[3][:, s:e], in_=x3_d[:, s:e])
    nc.sync.dma_start(out=xc[:, HXC:], in_=xc_d[:, HXC:])

    wq = consts.tile([128, WQF], DT)
    outf = consts.tile([128, QF], DT)

    # conv + SiLU pipelined with the MAC loop at pl-half granularity.
    xc3 = xc.rearrange("p (pl r) -> p pl r", pl=NPL)
    wq4 = wq.rearrange("p (pl k g) -> p pl k g", k=KK, g=G)
    NH = NPL // 2  # 32 cols per half
    HF = NH * GC * G  # 2048 free elems per half
    NCC = HF // 512  # 4 psum accumulator banks

    zpool = ctx.enter_context(tc.tile_pool(name="zp", bufs=2, space="PSUM"))
    shpool = ctx.enter_context(tc.tile_pool(name="shp", bufs=2, space="PSUM"))
    accpool = ctx.enter_context(tc.tile_pool(name="acc", bufs=1, space="PSUM"))

    def emit_shift(dh):
        d = dh - PAD
        lhs = ide[:, PAD + d : PAD + d + 128]
        for s in range(0, XT_F, 512):
            e = min(XT_F, s + 512)
            shp = shpool.tile([128, 512], f32, tag="shp")
            nc.tensor.matmul(shp[:, 0 : e - s], lhs, xt[3][:, s:e], start=True, stop=True)
            nc.scalar.copy(xt[dh][:, s:e], shp[:, 0 : e - s])

    for h in range(2):
        pl0 = h * NH
        for pl in range(pl0, pl0 + NH):
            z = zpool.tile([128, KO], f32, tag="z")
            nc.tensor.matmul(z, xc3[:, pl, :], wt, start=True, stop=True)
            nc.scalar.activation(wq[:, pl * KO : (pl + 1) * KO], z, silu)
        if h == 0:
            for dh in DH_ORDER:
                if dh != PAD:
                    emit_shift(dh)

        accs = [
            accpool.tile([128, 512], f32, tag=f"acc{cc}", name=f"acc{cc}")
            for cc in range(NCC)
        ]
        first = True
        for dh in DH_ORDER:
            for dw in range(K):
                k = dh * K + dw
                q = qpool.tile([128, HF], DT, tag="q")
                qv = q.rearrange("p (pl j g) -> p pl j g", j=GC, g=G)
                wv = wq4[:, pl0 : pl0 + NH, k : k + 1, :].broadcast_to(
                    [128, NH, GC, G]
                )
                xv = xt[dh][:, (pl0 + dw) * C : (pl0 + dw) * C + HF].rearrange(
                    "p (pl j g) -> p pl j g", j=GC, g=G
                )
                eng = nc.gpsimd if (dh in (4, 5) and dw in (2, 5)) else nc.vector
                eng.tensor_mul(qv, wv, xv)
                for cc in range(NCC):
                    nc.tensor.matmul(
                        accs[cc],
                        ide[:, PAD : PAD + 128],
                        q[:, cc * 512 : (cc + 1) * 512],
                        start=first,
                        stop=(dh == DH_ORDER[-1] and dw == K - 1),
                    )
                first = False
        for cc in range(NCC):
            sl = slice(h * HF + cc * 512, h * HF + (cc + 1) * 512)
            nc.scalar.copy(outf[:, sl], accs[cc])
            nc.sync.dma_start(out=out_d[:, sl], in_=outf[:, sl])
    ctx.close()


def prep_inputs(x, conv_w, bn_gamma, bn_beta, bn_mean, bn_var):
    """Host-side prep: per-core staged tensors (bf16)."""
    npdt = _npdt()
    scale = (bn_gamma / np.sqrt(bn_var + BN_EPS)).astype(np.float32)
    shift = (bn_beta - bn_mean * scale).astype(np.float32)

    # Wt[c, k*4+g] = conv_w[g*49+k, c] * scale ; row 64 = shift (bias row).
    o = (np.arange(G)[None, :] * KK + np.arange(KK)[:, None]).reshape(-1)  # (k,g)
    wt = np.zeros((C + 1, KO), npdt)
    wt[0:C] = (conv_w[o].T * scale[o][None, :]).astype(npdt)
    wt[C] = shift[o].astype(npdt)

    ide = np.zeros((128, 128 + 2 * PAD), npdt)
    ide[np.arange(128), np.arange(128) + PAD] = 1.0

    # channel slot c' = j*4+g holds channel g*16+j (g innermost keeps the
    # dw windows 4B-aligned).
    jj, gg = np.meshgrid(np.arange(GC), np.arange(G), indexing="ij")
    perm = (gg * GC + jj).reshape(-1)

    in_maps = []
    for core in range(8):
        b, wh = divmod(core, 2)
        c0 = NPL * wh
        xc = np.ones((C + 1, NPL, H), npdt)
        xc[0:C] = x[b, :, :, c0 : c0 + NPL].transpose(0, 2, 1).astype(npdt)

        xpadW = np.zeros((C, H, W + 2 * PAD), npdt)
        xpadW[:, :, PAD : PAD + W] = x[b, perm].astype(npdt)
        x3 = xpadW[:, :, c0 : c0 + XT_COLS].transpose(1, 2, 0)  # [r, t, c']

        in_maps.append(
            {
                "xc": xc.reshape(C + 1, NPL * H),
                "wt": wt,
                "ide": ide,
                "x3": np.ascontiguousarray(x3).reshape(H, XT_F),
            }
        )
    return in_maps


def assemble_output(results):
    out = np.zeros((B, C, H, W), np.float32)
    for core in range(8):
        b, wh = divmod(core, 2)
        c0 = NPL * wh
        oc = np.asarray(results[core]["out"], np.float32).reshape(H, NPL, GC, G)
        out[b, :, :, c0 : c0 + NPL] = oc.transpose(3, 2, 0, 1).reshape(C, H, NPL)
    return out


def kernel(x, conv_w, bn_gamma, bn_beta, bn_mean, bn_var):
    x = np.asarray(x, np.float32)
    conv_w = np.asarray(conv_w, np.float32)
    in_maps = prep_inputs(
        x,
        conv_w,
        np.asarray(bn_gamma, np.float32),
        np.asarray(bn_beta, np.float32),
        np.asarray(bn_mean, np.float32),
        np.asarray(bn_var, np.float32),
    )
    nc = build_bass()
    res = run_bass_kernel_spmd(nc, in_maps, core_ids=list(range(8)))
    return assemble_output(res.results)


if __name__ == "__main__":
    rng = np.random.default_rng(0)
    ins = {
        "x": rng.standard_normal((B, C, H, W), np.float32),
        "conv_w": rng.standard_normal((KO, C), np.float32) / 8.0,
        "bn_gamma": rng.uniform(0.5, 1.5, KO).astype(np.float32),
        "bn_beta": rng.standard_normal(KO).astype(np.float32) * 0.1,
        "bn_mean": rng.standard_normal(KO).astype(np.float32) * 0.1,
        "bn_var": rng.uniform(0.5, 1.5, KO).astype(np.float32),
    }
    out = kernel(**ins)
    print("kernel output", out.shape, out.dtype, np.abs(out).sum())


# revision 7
# speedup vs baseline: 1.0454x; 1.0454x over previous
"""Trainium2 Bass kernel for the Involution module (B=4, C=64, H=W=128, K=7, G=4).

v3 = v2 (pixel-partition layout, stride-0 broadcast muls, PE accumulation)
plus:
  - only one x copy (dh=3, natural rows) is loaded from HBM; the 6
    row-shifted copies are built on-chip with partition-shifted
    SBUF->SBUF DMAs (cuts HBM input ~4x; the v2 trace showed the muls
    stalling ~6-8us per dh group on contended HBM loads).
  - k-loop runs dh=3 first so its muls start while the shifted copies
    are still being built.
  - conv/SiLU and the MAC loop are pipelined at pl-half granularity
    (cuts the serial prologue roughly in half); 4 shared PSUM
    accumulator banks + 4 conv banks fit PSUM exactly.
  - output returned as bf16 (halves output DMA).
"""

import os

os.environ.setdefault("JAX_PLATFORMS", "cpu")

import numpy as np
import ml_dtypes

import concourse.bacc as bacc
import concourse.tile as tile
import concourse.mybir as mybir
from concourse.bass_utils import run_bass_kernel_spmd

# Problem constants (hardcoded per harness contract).
B, C, H, W = 4, 64, 128, 128
K, G, GC = 7, 4, 16
KK = K * K
KO = KK * G  # 196
PAD = 3
BN_EPS = 1e-5

NPL = 64  # output cols per core (W-half)
XT_COLS = NPL + 2 * PAD  # 70 stored cols (halo included)
XT_F = XT_COLS * C  # 4480 free elems per dh-copy
QF = NPL * GC * G  # 4096 free elems of out
WQF = NPL * KO  # 12544 free elems of w'

DH_ORDER = [3, 2, 4, 1, 5, 0, 6]

USE_BF16 = True


def _dt():
    return mybir.dt.bfloat16 if USE_BF16 else mybir.dt.float32


def _npdt():
    return ml_dtypes.bfloat16 if USE_BF16 else np.float32


def build_bass():
    nc = bacc.Bacc(
        "TRN2",
        target_bir_lowering=False,
        debug=False,
        enable_asserts=False,
        num_devices=8,
    )
    DT = _dt()

    xc_d = nc.dram_tensor("xc", [C + 1, NPL * H], DT, kind="ExternalInput").ap()
    wt_d = nc.dram_tensor("wt", [C + 1, KO], DT, kind="ExternalInput").ap()
    id_d = nc.dram_tensor("ide", [128, 128 + 2 * PAD], DT, kind="ExternalInput").ap()
    x3_d = nc.dram_tensor("x3", [128, XT_F], DT, kind="ExternalInput").ap()
    out_d = nc.dram_tensor("out", [128, QF], DT, kind="ExternalOutput").ap()

    with tile.TileContext(nc) as tc:
        build_kernel(tc, xc_d, wt_d, id_d, x3_d, out_d)
    nc.compile()
    return nc


def build_kernel(tc, xc_d, wt_d, id_d, x3_d, out_d):
    from contextlib import ExitStack

    nc = tc.nc
    DT = _dt()
    f32 = mybir.dt.float32
    silu = mybir.ActivationFunctionType.Silu

    ctx = ExitStack()
    consts = ctx.enter_context(tc.tile_pool(name="consts", bufs=1))
    qpool = ctx.enter_context(tc.tile_pool(name="q", bufs=3))

    wt = consts.tile([C + 1, KO], DT)
    nc.sync.dma_start(out=wt, in_=wt_d)
    xc = consts.tile([C + 1, NPL * H], DT)
    # ide[p, p+3] = 1: slice [:, 3+d : 131+d] is the row-shift-by-d
    # matrix (out[m] = in[m+d], zeros where out of range); [:, 3:131]
    # is the plain identity used by the accumulate matmuls.
    ide = consts.tile([128, 128 + 2 * PAD], DT)
    nc.sync.dma_start(out=ide, in_=id_d)

    # xt[dh][r, (t, c)] = x[perm[c], r+dh-3, c0-3+t]; dh=3 from HBM, the
    # rest built by PE row-shift matmuls (boundary rows zero for free).
    xt = [None] * K
    for dh in range(K):
        t = consts.tile([128, XT_F], DT, tag="", name=f"xts{dh}")
        xt[dh] = t
    # Prioritized, chunked input loads: the first conv block gates the
    # whole pipeline, then the x3 window its muls need, then the rest.
    QXC = NPL * H // 4
    nc.sync.dma_start(out=xc[:, 0:QXC], in_=xc_d[:, 0:QXC])
    nc.sync.dma_start(out=xt[3][:, 0:1536], in_=x3_d[:, 0:1536])
    nc.sync.dma_start(out=xc[:, QXC : 2 * QXC], in_=xc_d[:, QXC : 2 * QXC])
    for s in range(1536, XT_F, 1536):
        e = min(XT_F, s + 1536)
        nc.sync.dma_start(out=xt[3][:, s:e], in_=x3_d[:, s:e])
    nc.sync.dma_start(out=xc[:, 2 * QXC :], in_=xc_d[:, 2 * QXC :])

    wq = consts.tile([128, WQF], DT)
    outf = consts.tile([128, QF], DT)

    # conv + SiLU pipelined with the MAC loop at pl-block granularity
    # (16 / 32 / 16 cols: short ramp-in and ramp-out, same DVE volume).
    xc3 = xc.rearrange("p (pl r) -> p pl r", pl=NPL)
    wq4 = wq.rearrange("p (pl k g) -> p pl k g", k=KK, g=G)
    BLOCKS = [(0, 16), (16, 32), (48, 16)]

    zpool = ctx.enter_context(tc.tile_pool(name="zp", bufs=2, space="PSUM"))
    shpool = ctx.enter_context(tc.tile_pool(name="shp", bufs=2, space="PSUM"))
    accpool = ctx.enter_context(tc.tile_pool(name="acc", bufs=1, space="PSUM"))

    def emit_shift(dh):
        d = dh - PAD
        lhs = ide[:, PAD + d : PAD + d + 128]
        for s in range(0, XT_F, 512):
            e = min(XT_F, s + 512)
            shp = shpool.tile([128, 512], f32, tag="shp")
            nc.tensor.matmul(shp[:, 0 : e - s], lhs, xt[3][:, s:e], start=True, stop=True)
            nc.scalar.copy(xt[dh][:, s:e], shp[:, 0 : e - s])

    for bi, (pl0, npl) in enumerate(BLOCKS):
        for pl in range(pl0, pl0 + npl):
            z = zpool.tile([128, KO], f32, tag="z")
            nc.tensor.matmul(z, xc3[:, pl, :], wt, start=True, stop=True)
            nc.scalar.activation(wq[:, pl * KO : (pl + 1) * KO], z, silu)
        if bi == 0:
            for dh in DH_ORDER:
                if dh != PAD:
                    emit_shift(dh)

        bf = npl * GC * G
        ncc = bf // 512
        accs = [
            accpool.tile([128, 512], f32, tag=f"acc{cc}", name=f"acc{cc}")
            for cc in range(ncc)
        ]
        first = True
        for dh in DH_ORDER:
            for dw in range(K):
                k = dh * K + dw
                q = qpool.tile([128, bf], DT, tag="q", name="q")
                qv = q.rearrange("p (pl j g) -> p pl j g", j=GC, g=G)
                wv = wq4[:, pl0 : pl0 + npl, k : k + 1, :].broadcast_to(
                    [128, npl, GC, G]
                )
                xv = xt[dh][:, (pl0 + dw) * C : (pl0 + dw) * C + bf].rearrange(
                    "p (pl j g) -> p pl j g", j=GC, g=G
                )
                nc.vector.tensor_mul(qv, wv, xv)
                for cc in range(ncc):
                    nc.tensor.matmul(
                        accs[cc],
                        ide[:, PAD : PAD + 128],
                        q[:, cc * 512 : (cc + 1) * 512],
                        start=first,
                        stop=(dh == DH_ORDER[-1] and dw == K - 1),
                    )
                first = False
        for cc in range(ncc):
            sl = slice(pl0 * GC * G + cc * 512, pl0 * GC * G + (cc + 1) * 512)
            nc.scalar.copy(outf[:, sl], accs[cc])
            nc.sync.dma_start(out=out_d[:, sl], in_=outf[:, sl])
    ctx.close()


def prep_inputs(x, conv_w, bn_gamma, bn_beta, bn_mean, bn_var):
    """Host-side prep: per-core staged tensors (bf16)."""
    npdt = _npdt()
    scale = (bn_gamma / np.sqrt(bn_var + BN_EPS)).astype(np.float32)
    shift = (bn_beta - bn_mean * scale).astype(np.float32)

    # Wt[c, k*4+g] = conv_w[g*49+k, c] * scale ; row 64 = shift (bias row).
    o = (np.arange(G)[None, :] * KK + np.arange(KK)[:, None]).reshape(-1)  # (k,g)
    wt = np.zeros((C + 1, KO), npdt)
    wt[0:C] = (conv_w[o].T * scale[o][None, :]).astype(npdt)
    wt[C] = shift[o].astype(npdt)

    ide = np.zeros((128, 128 + 2 * PAD), npdt)
    ide[np.arange(128), np.arange(128) + PAD] = 1.0

    # channel slot c' = j*4+g holds channel g*16+j (g innermost keeps the
    # dw windows 4B-aligned).
    jj, gg = np.meshgrid(np.arange(GC), np.arange(G), indexing="ij")
    perm = (gg * GC + jj).reshape(-1)

    in_maps = []
    for core in range(8):
        b, wh = divmod(core, 2)
        c0 = NPL * wh
        xc = np.ones((C + 1, NPL, H), npdt)
        xc[0:C] = x[b, :, :, c0 : c0 + NPL].transpose(0, 2, 1).astype(npdt)

        xpadW = np.zeros((C, H, W + 2 * PAD), npdt)
        xpadW[:, :, PAD : PAD + W] = x[b, perm].astype(npdt)
        x3 = xpadW[:, :, c0 : c0 + XT_COLS].transpose(1, 2, 0)  # [r, t, c']

        in_maps.append(
            {
                "xc": xc.reshape(C + 1, NPL * H),
                "wt": wt,
                "ide": ide,
                "x3": np.ascontiguousarray(x3).reshape(H, XT_F),
            }
        )
    return in_maps


def assemble_output(results):
    out = np.zeros((B, C, H, W), np.float32)
    for core in range(8):
        b, wh = divmod(core, 2)
        c0 = NPL * wh
        oc = np.asarray(results[core]["out"], np.float32).reshape(H, NPL, GC, G)
        out[b, :, :, c0 : c0 + NPL] = oc.transpose(3, 2, 0, 1).reshape(C, H, NPL)
    return out


def kernel(x, conv_w, bn_gamma, bn_beta, bn_mean, bn_var):
    x = np.asarray(x, np.float32)
    conv_w = np.asarray(conv_w, np.float32)
    in_maps = prep_inputs(
        x,
        conv_w,
        np.asarray(bn_gamma, np.float32),
        np.asarray(bn_beta, np.float32),
        np.asarray(bn_mean, np.float32),
        np.asarray(bn_var, np.float32),
    )
    nc = build_bass()
    res = run_bass_kernel_spmd(nc, in_maps, core_ids=list(range(8)))
    return assemble_output(res.results)


if __name__ == "__main__":
    rng = np.random.default_rng(0)
    ins = {
        "x": rng.standard_normal((B, C, H, W), np.float32),
        "conv_w": rng.standard_normal((KO, C), np.float32) / 8.0,
        "bn_gamma": rng.uniform(0.5, 1.5, KO).astype(np.float32),
        "bn_beta": rng.standard_normal(KO).astype(np.float32) * 0.1,
        "bn_mean": rng.standard_normal(KO).astype(np.float32) * 0.1,
        "bn_var": rng.uniform(0.5, 1.5, KO).astype(np.float32),
    }
    out = kernel(**ins)
    print("kernel output", out.shape, out.dtype, np.abs(out).sum())


# revision 8
# speedup vs baseline: 1.2031x; 1.1508x over previous
"""Trainium2 Bass kernel for the Involution module (B=4, C=64, H=W=128, K=7, G=4).

v3 = v2 (pixel-partition layout, stride-0 broadcast muls, PE accumulation)
plus:
  - only one x copy (dh=3, natural rows) is loaded from HBM; the 6
    row-shifted copies are built on-chip with partition-shifted
    SBUF->SBUF DMAs (cuts HBM input ~4x; the v2 trace showed the muls
    stalling ~6-8us per dh group on contended HBM loads).
  - k-loop runs dh=3 first so its muls start while the shifted copies
    are still being built.
  - conv/SiLU and the MAC loop are pipelined at pl-half granularity
    (cuts the serial prologue roughly in half); 4 shared PSUM
    accumulator banks + 4 conv banks fit PSUM exactly.
  - output returned as bf16 (halves output DMA).
"""

import os

os.environ.setdefault("JAX_PLATFORMS", "cpu")

import numpy as np
import ml_dtypes

import concourse.bacc as bacc
import concourse.tile as tile
import concourse.mybir as mybir
from concourse.bass_utils import run_bass_kernel_spmd

# Problem constants (hardcoded per harness contract).
B, C, H, W = 4, 64, 128, 128
K, G, GC = 7, 4, 16
KK = K * K
KO = KK * G  # 196
PAD = 3
BN_EPS = 1e-5

NPL = 64  # output cols per core (W-half)
XT_COLS = NPL + 2 * PAD  # 70 stored cols (halo included)
XT_F = XT_COLS * C  # 4480 free elems per dh-copy
QF = NPL * GC * G  # 4096 free elems of out
WQF = NPL * KO  # 12544 free elems of w'

DH_ORDER = [3, 2, 4, 1, 5, 0, 6]

USE_BF16 = True


def _dt():
    return mybir.dt.bfloat16 if USE_BF16 else mybir.dt.float32


def _npdt():
    return ml_dtypes.bfloat16 if USE_BF16 else np.float32


def build_bass():
    nc = bacc.Bacc(
        "TRN2",
        target_bir_lowering=False,
        debug=False,
        enable_asserts=False,
        num_devices=8,
    )
    DT = _dt()

    xc_d = nc.dram_tensor("xc", [C + 1, NPL * H], DT, kind="ExternalInput").ap()
    wt_d = nc.dram_tensor("wt", [C + 1, KO], DT, kind="ExternalInput").ap()
    id_d = nc.dram_tensor("ide", [128, 128], DT, kind="ExternalInput").ap()
    xt_d = [
        nc.dram_tensor(f"xt{dh}", [128, XT_F], DT, kind="ExternalInput").ap()
        for dh in range(K)
    ]
    out_d = nc.dram_tensor("out", [128, QF], DT, kind="ExternalOutput").ap()

    with tile.TileContext(nc) as tc:
        build_kernel(tc, xc_d, wt_d, id_d, xt_d, out_d)
    nc.compile()
    return nc


def build_kernel(tc, xc_d, wt_d, id_d, xt_d, out_d):
    from contextlib import ExitStack

    nc = tc.nc
    DT = _dt()
    f32 = mybir.dt.float32
    silu = mybir.ActivationFunctionType.Silu

    ctx = ExitStack()
    consts = ctx.enter_context(tc.tile_pool(name="consts", bufs=1))
    qpool = ctx.enter_context(tc.tile_pool(name="q", bufs=3))

    wt = consts.tile([C + 1, KO], DT)
    nc.sync.dma_start(out=wt, in_=wt_d)
    xc = consts.tile([C + 1, NPL * H], DT)
    ide = consts.tile([128, 128], DT)
    nc.sync.dma_start(out=ide, in_=id_d)

    # xt[dh][r, (t, c)] = x[perm[c], r+dh-3, c0-3+t], host-staged; loads
    # issued in DH_ORDER so each arrives before its muls (the first conv
    # block's xc slice goes first — it gates the whole pipeline).
    xt = [None] * K
    for dh in range(K):
        t = consts.tile([128, XT_F], DT, tag="", name=f"xts{dh}")
        xt[dh] = t
    QXC = NPL * H // 4
    nc.sync.dma_start(out=xc[:, 0:QXC], in_=xc_d[:, 0:QXC])
    nc.sync.dma_start(out=xt[3], in_=xt_d[3])
    nc.sync.dma_start(out=xc[:, QXC : 2 * QXC], in_=xc_d[:, QXC : 2 * QXC])
    nc.sync.dma_start(out=xt[2], in_=xt_d[2])
    nc.sync.dma_start(out=xt[4], in_=xt_d[4])
    nc.sync.dma_start(out=xc[:, 2 * QXC :], in_=xc_d[:, 2 * QXC :])
    for dh in (1, 5, 0, 6):
        nc.sync.dma_start(out=xt[dh], in_=xt_d[dh])

    wq = consts.tile([128, WQF], DT)
    outf = consts.tile([128, QF], DT)

    # conv + SiLU pipelined with the MAC loop at pl-block granularity
    # (16 / 32 / 16 cols: short ramp-in and ramp-out, same DVE volume).
    xc3 = xc.rearrange("p (pl r) -> p pl r", pl=NPL)
    wq4 = wq.rearrange("p (pl k g) -> p pl k g", k=KK, g=G)
    BLOCKS = [(0, 16), (16, 32), (48, 16)]

    zpool = ctx.enter_context(tc.tile_pool(name="zp", bufs=4, space="PSUM"))
    accpool = ctx.enter_context(tc.tile_pool(name="acc", bufs=1, space="PSUM"))

    for bi, (pl0, npl) in enumerate(BLOCKS):
        for pl in range(pl0, pl0 + npl):
            z = zpool.tile([128, KO], f32, tag="z")
            nc.tensor.matmul(z, xc3[:, pl, :], wt, start=True, stop=True)
            nc.scalar.activation(wq[:, pl * KO : (pl + 1) * KO], z, silu)

        bf = npl * GC * G
        ncc = bf // 512
        accs = [
            accpool.tile([128, 512], f32, tag=f"acc{cc}", name=f"acc{cc}")
            for cc in range(ncc)
        ]
        first = True
        for dh in DH_ORDER:
            for dw in range(K):
                k = dh * K + dw
                q = qpool.tile([128, bf], DT, tag="q", name="q")
                qv = q.rearrange("p (pl j g) -> p pl j g", j=GC, g=G)
                wv = wq4[:, pl0 : pl0 + npl, k : k + 1, :].broadcast_to(
                    [128, npl, GC, G]
                )
                xv = xt[dh][:, (pl0 + dw) * C : (pl0 + dw) * C + bf].rearrange(
                    "p (pl j g) -> p pl j g", j=GC, g=G
                )
                nc.vector.tensor_mul(qv, wv, xv)
                for cc in range(ncc):
                    nc.tensor.matmul(
                        accs[cc],
                        ide,
                        q[:, cc * 512 : (cc + 1) * 512],
                        start=first,
                        stop=(dh == DH_ORDER[-1] and dw == K - 1),
                    )
                first = False
        for cc in range(ncc):
            sl = slice(pl0 * GC * G + cc * 512, pl0 * GC * G + (cc + 1) * 512)
            nc.scalar.copy(outf[:, sl], accs[cc])
            nc.sync.dma_start(out=out_d[:, sl], in_=outf[:, sl])
    ctx.close()


def prep_inputs(x, conv_w, bn_gamma, bn_beta, bn_mean, bn_var):
    """Host-side prep: per-core staged tensors (bf16)."""
    npdt = _npdt()
    scale = (bn_gamma / np.sqrt(bn_var + BN_EPS)).astype(np.float32)
    shift = (bn_beta - bn_mean * scale).astype(np.float32)

    # Wt[c, k*4+g] = conv_w[g*49+k, c] * scale ; row 64 = shift (bias row).
    o = (np.arange(G)[None, :] * KK + np.arange(KK)[:, None]).reshape(-1)  # (k,g)
    wt = np.zeros((C + 1, KO), npdt)
    wt[0:C] = (conv_w[o].T * scale[o][None, :]).astype(npdt)
    wt[C] = shift[o].astype(npdt)

    ide = np.eye(128, dtype=npdt)

    # channel slot c' = j*4+g holds channel g*16+j (g innermost keeps the
    # dw windows 4B-aligned).
    jj, gg = np.meshgrid(np.arange(GC), np.arange(G), indexing="ij")
    perm = (gg * GC + jj).reshape(-1)

    in_maps = []
    for core in range(8):
        b, wh = divmod(core, 2)
        c0 = NPL * wh
        xc = np.ones((C + 1, NPL, H), npdt)
        xc[0:C] = x[b, :, :, c0 : c0 + NPL].transpose(0, 2, 1).astype(npdt)

        xpadW = np.zeros((C, H, W + 2 * PAD), npdt)
        xpadW[:, :, PAD : PAD + W] = x[b, perm].astype(npdt)
        m = {"xc": xc.reshape(C + 1, NPL * H), "wt": wt, "ide": ide}
        for dh in range(K):
            t = np.zeros((H, XT_COLS, C), npdt)
            rlo, rhi = max(0, PAD - dh), min(H, H + PAD - dh)
            t[rlo:rhi] = xpadW[
                :, rlo + dh - PAD : rhi + dh - PAD, c0 : c0 + XT_COLS
            ].transpose(1, 2, 0)
            m[f"xt{dh}"] = np.ascontiguousarray(t).reshape(H, XT_F)
        in_maps.append(m)
    return in_maps


def assemble_output(results):
    out = np.zeros((B, C, H, W), np.float32)
    for core in range(8):
        b, wh = divmod(core, 2)
        c0 = NPL * wh
        oc = np.asarray(results[core]["out"], np.float32).reshape(H, NPL, GC, G)
        out[b, :, :, c0 : c0 + NPL] = oc.transpose(3, 2, 0, 1).reshape(C, H, NPL)
    return out


def kernel(x, conv_w, bn_gamma, bn_beta, bn_mean, bn_var):
    x = np.asarray(x, np.float32)
    conv_w = np.asarray(conv_w, np.float32)
    in_maps = prep_inputs(
        x,
        conv_w,
        np.asarray(bn_gamma, np.float32),
        np.asarray(bn_beta, np.float32),
        np.asarray(bn_mean, np.float32),
        np.asarray(bn_var, np.float32),
    )
    nc = build_bass()
    res = run_bass_kernel_spmd(nc, in_maps, core_ids=list(range(8)))
    return assemble_output(res.results)


if __name__ == "__main__":
    rng = np.random.default_rng(0)
    ins = {
        "x": rng.standard_normal((B, C, H, W), np.float32),
        "conv_w": rng.standard_normal((KO, C), np.float32) / 8.0,
        "bn_gamma": rng.uniform(0.5, 1.5, KO).astype(np.float32),
        "bn_beta": rng.standard_normal(KO).astype(np.float32) * 0.1,
        "bn_mean": rng.standard_normal(KO).astype(np.float32) * 0.1,
        "bn_var": rng.uniform(0.5, 1.5, KO).astype(np.float32),
    }
    out = kernel(**ins)
    print("kernel output", out.shape, out.dtype, np.abs(out).sum())
